# revision 33
# baseline (speedup 1.0000x reference)
"""Trainium2 Bass kernel for nn_AttentionHead_15805479649975.

Single attention head, B=8, S=2048, d_model=1024, d_k=64.
  Q = x@Wq+bq; K = x@Wk+bk; V = x@Wv+bv
  scores = Q K^T / 8; P = softmax(scores)            (full-row softmax)
  P = P * tril * (pm outer pm)                        (masks applied AFTER softmax)
  out = P V

Sharding: data-parallel over the batch dim — one batch element per NeuronCore,
projection weights replicated, no cross-core communication.

Per-core device algorithm (everything in fp32):
  Phase 1: stream xT (host-pretransposed x[b].T) in 8 d-chunks; compute
    QT/KT via one fused [Wq|Wk] matmul (M=128) and VT (M=64); PE-transpose
    VT into natural V chunks, scale by pm[k], append a ones column -> Vaug.
  Phase 2 (two q-halves of 1024 to fit PSUM): for each key tile j compute
    ST[k,q] = K Q^T (K=64 matmul), exp on ScalarE (scale=1/8 fused, no
    max-subtraction needed: |scores/8| is O(1)), then accumulate
      attnT[0:64, q] += V'[k,:]^T P^T[k, q]   (causal part, via Vaug)
      attnT[64,  q] += sum_k exp(..)          (softmax denominator, via the
                                               ones column / a zero|ones lhsT
                                               for the non-causal columns)
    Finally out[0:64] *= pm[q] / attnT[64] via reciprocal + gpsimd
    partition-broadcast, and DMA attnT back; host transposes to [S, 64].
"""

import math
from contextlib import ExitStack

import numpy as np

import concourse.bacc as bacc
import concourse.bass as bass
import concourse.mybir as mybir
import concourse.tile as tile
from concourse.bass_utils import run_bass_kernel_spmd

B, S, D, DK = 8, 2048, 1024, 64
NCORES = 8
FP = mybir.dt.float32
NT = S // 128          # 16 key tiles
NH = 2                 # q halves
HQ = S // NH           # 1024 queries per half
DC = D // 128          # 8 contraction chunks

# misc layout: col 0 = [bq; bk], col 1 = bv (rows 0:64), cols 2:18 = pmT,
# cols 18:146 = identity(128), cols 146:274 = tril mask (k<=q)
MISC_BQK = 0
MISC_BV = 1
MISC_PMT = 2
MISC_ID = 18
MISC_TRI = 146
MISC_W = 274

AF = mybir.ActivationFunctionType
MUL = mybir.AluOpType.mult

# float32r: single-pass "relaxed" fp32 matmul (TF32-ish). Plain fp32 runs as
# 2 half-speed passes (4x cycles/row) on TRN2's PE; fp32r streams at 1x.
USE_F32R = True
FR = mybir.dt.float32r if USE_F32R else mybir.dt.float32


def _mmcast(ap):
    return ap


def _chunks(lo, hi, step=512):
    """Split [lo, hi) at the 512-column grid (PSUM bank boundaries).

    Bank alignment matters: a matmul with start=True clears the has_written
    bits of every bank it touches, so no chunk may straddle a bank whose
    prior accumulations must survive."""
    out = []
    while lo < hi:
        nxt = min((lo // step + 1) * step, hi)
        out.append((lo, nxt))
        lo = nxt
    return out


def build_body(ctx: ExitStack, tc, xT, wqk, wv, misc, pmrow, z1d, onesr, onescols, out, dbg=None):
    nc = tc.nc

    consts = ctx.enter_context(tc.tile_pool(name="consts", bufs=1))
    xpool = ctx.enter_context(tc.tile_pool(name="x", bufs=DC))
    proj = ctx.enter_context(tc.tile_pool(name="proj", bufs=1))
    ppool = ctx.enter_context(tc.tile_pool(name="pt", bufs=3))
    dpool = ctx.enter_context(tc.tile_pool(name="ptd", bufs=2))
    fin = ctx.enter_context(tc.tile_pool(name="fin", bufs=2))

    # ---- constants (weights first; everything not needed until later is
    # DMA'd after the x chunks so x transfers start as early as possible)
    wqk_s = consts.tile([128, DC, 128], FR)
    nc.sync.dma_start(wqk_s[:], wqk[:])
    wv_s = consts.tile([128, DC, DK], FR)
    nc.sync.dma_start(wv_s[:], wv[:])

    # ---- x chunks (all resident; DMA-bound phase)
    xs = []
    for c in range(DC):
        xc = xpool.tile([128, S], FR, tag="xc")
        nc.sync.dma_start(xc[:], xT[c * 128 : (c + 1) * 128, :])
        xs.append(xc)

    misc_s = consts.tile([128, MISC_W], FP)
    nc.sync.dma_start(misc_s[:], misc[:])
    pmrow_s = consts.tile([1, S], FP)
    nc.sync.dma_start(pmrow_s[:], pmrow[:])
    z1_s = consts.tile([128, DK + 1], FR)
    nc.sync.dma_start(z1_s[:], z1d[:])
    ones_row = consts.tile([1, DK], FR)
    nc.sync.dma_start(ones_row[:], onesr[:])

    qt_s = proj.tile([DK, S], FR, tag="qt")
    kt_s = proj.tile([DK, S], FR, tag="kt")
    vt_s = proj.tile([DK, S], FP, tag="vt")
    vaug_s = proj.tile([128, NT * (DK + 1)], FR, tag="vaug")
    attn_s = proj.tile([DK, S], FP, tag="attn")
    # ones columns of vaug (one strided DMA; memset can't write f32r)
    vaug_cols = vaug_s[:].rearrange("p (t c) -> p t c", c=DK + 1)[:, :, DK : DK + 1]
    nc.sync.dma_start(vaug_cols, onescols[:].rearrange("p (t o) -> p t o", o=1))

    ident = misc_s[0:DK, MISC_ID : MISC_ID + DK]

    # ---- phase 1: projections
    with tc.tile_pool(name="p1", bufs=2, space="PSUM") as p1:
        for n in range(4):
            nlo = n * 512
            qk_ps = p1.tile([128, 512], FP, tag="qk")
            for c in range(DC):
                nc.tensor.matmul(
                    qk_ps[:], _mmcast(wqk_s[:, c, :]),
                    _mmcast(xs[c][:, nlo : nlo + 512]),
                    start=(c == 0), stop=(c == DC - 1),
                )
            nc.vector.tensor_scalar_add(
                qt_s[:, nlo : nlo + 512], qk_ps[0:DK, :],
                misc_s[0:DK, MISC_BQK : MISC_BQK + 1],
            )
            nc.vector.tensor_scalar_add(
                kt_s[:, nlo : nlo + 512], qk_ps[DK:128, :],
                misc_s[DK:128, MISC_BQK : MISC_BQK + 1],
            )
            v_ps = p1.tile([DK, 512], FP, tag="v")
            for c in range(DC):
                nc.tensor.matmul(
                    v_ps[:], _mmcast(wv_s[:, c, :]),
                    _mmcast(xs[c][:, nlo : nlo + 512]),
                    start=(c == 0), stop=(c == DC - 1),
                )
            nc.vector.tensor_scalar_add(
                vt_s[:, nlo : nlo + 512], v_ps[:],
                misc_s[0:DK, MISC_BV : MISC_BV + 1],
            )
        # VT -> V natural chunks, scaled by pm[k]; ones column appended
        for j in range(NT):
            tr_ps = p1.tile([128, DK], FP, tag="tr")
            nc.tensor.transpose(tr_ps[:], vt_s[:, j * 128 : (j + 1) * 128], ident)
            nc.vector.tensor_scalar_mul(
                vaug_s[:, j * 65 : j * 65 + DK], tr_ps[:],
                misc_s[:, MISC_PMT + j : MISC_PMT + j + 1],
            )

    # ---- phase 2: scores/softmax/PV per q-half
    with (
        tc.tile_pool(name="p2st", bufs=2, space="PSUM") as p2st,
        tc.tile_pool(name="p2att", bufs=1, space="PSUM") as p2att,
    ):
        for h in range(NH):
            qlo = h * HQ
            att_ps = p2att.tile([DK + 1, HQ], FP, tag="att")
            bank_cleared = [False] * (HQ // 512)

            def pv_mm(lo, hi, lhsT, rhs, part_hi, stop=False, rhs_off=0):
                for clo, chi in _chunks(lo, hi):
                    bank = clo // 512
                    nc.tensor.matmul(
                        att_ps[0:part_hi, clo:chi], _mmcast(lhsT),
                        _mmcast(rhs[:, clo - rhs_off : chi - rhs_off]),
                        start=not bank_cleared[bank], stop=stop,
                    )
                    bank_cleared[bank] = True

            for j in range(NT):
                st_ps = p2st.tile([128, HQ], FP, tag="st")
                for lo, hi in _chunks(0, HQ):
                    nc.tensor.matmul(
                        st_ps[:, lo:hi],
                        _mmcast(kt_s[:, j * 128 : (j + 1) * 128]),
                        _mmcast(qt_s[:, qlo + lo : qlo + hi]),
                        start=True, stop=True,
                    )
                pt = ppool.tile([128, HQ], FR, tag="pt")
                nc.scalar.activation(pt[:], st_ps[:], AF.Exp, scale=1.0 / math.sqrt(DK))

                split = min(max(128 * (j + 1) - qlo, 0), HQ)
                # diagonal block: tril-masked V-part (emitted first so it owns
                # the bank-clear when it is the first writer)
                dlo = 128 * j - qlo
                if 0 <= dlo < HQ:
                    ptd = dpool.tile([128, 128], FR, tag="ptd")
                    nc.vector.tensor_tensor(
                        ptd[:], pt[:, dlo : dlo + 128].bitcast(FP),
                        misc_s[:, MISC_TRI : MISC_TRI + 128], MUL,
                    )
                    pv_mm(dlo, dlo + 128, vaug_s[:, j * 65 : j * 65 + DK], ptd,
                          part_hi=DK, rhs_off=dlo)
                # denominator for columns q < 128*(j+1): zero|ones lhsT
                pv_mm(0, split, z1_s[:], pt, part_hi=DK + 1, stop=(j == NT - 1))
                # strictly-causal columns: augmented [pm*V | 1]
                pv_mm(split, HQ, vaug_s[:, j * 65 : (j + 1) * 65], pt,
                      part_hi=DK + 1)
            # tail: out = attnT[0:64] * (pm[q] / denom[q])
            rcp = fin.tile([1, HQ], FP, tag="rcp")
            nc.vector.reciprocal(rcp[:], att_ps[DK : DK + 1, :])
            rfin = fin.tile([1, HQ], FR, tag="rfin")
            nc.vector.tensor_tensor(rfin[:], rcp[:], pmrow_s[:, qlo : qlo + HQ], MUL)
            # broadcast rfin across 64 partitions: K=1 outer product on PE
            bc_full = p2st.tile([128, HQ], FP, tag="st")
            bc_ps = bc_full[0:DK, :]
            for lo, hi in _chunks(0, HQ):
                nc.tensor.matmul(
                    bc_ps[:, lo:hi], _mmcast(ones_row[:]), _mmcast(rfin[:, lo:hi]),
                    start=True, stop=True,
                )
            bc_s = fin.tile([DK, HQ], FP, tag="bc")
            nc.vector.tensor_scalar_mul(bc_s[:], bc_ps[:], 1.0)
            nc.vector.tensor_tensor(
                attn_s[:, qlo : qlo + HQ], att_ps[0:DK, :], bc_s[:], MUL
            )
            nc.sync.dma_start(out[:, qlo : qlo + HQ], attn_s[:, qlo : qlo + HQ])
            if dbg is not None:
                den_s = fin.tile([1, HQ], FP, tag="dens")
                nc.vector.tensor_scalar_mul(den_s[:], att_ps[DK : DK + 1, :], 1.0)
                nc.sync.dma_start(dbg["den"][:, qlo : qlo + HQ], den_s[:])
                num_s = fin.tile([DK, HQ], FP, tag="nums")
                nc.vector.tensor_scalar_mul(num_s[:], att_ps[0:DK, :], 1.0)
                nc.sync.dma_start(dbg["num"][:, qlo : qlo + HQ], num_s[:])
    if dbg is not None:
        nc.sync.dma_start(dbg["qt"][:], qt_s[:].bitcast(FP))
        nc.sync.dma_start(dbg["kt"][:], kt_s[:].bitcast(FP))
        nc.sync.dma_start(dbg["vaug"][:], vaug_s[:].bitcast(FP))


_NC = {}


def build_nc(debug_outputs=False):
    if debug_outputs in _NC:
        return _NC[debug_outputs]
    nc = bacc.Bacc("TRN2", target_bir_lowering=False, debug=False, num_devices=NCORES)
    xT = nc.declare_dram_parameter("xT", [D, S], FR, isOutput=False)
    wqk = nc.declare_dram_parameter("wqk", [128, DC, 128], FR, isOutput=False)
    wv = nc.declare_dram_parameter("wv", [128, DC, DK], FR, isOutput=False)
    misc = nc.declare_dram_parameter("misc", [128, MISC_W], FP, isOutput=False)
    pmrow = nc.declare_dram_parameter("pmrow", [1, S], FP, isOutput=False)
    z1d = nc.declare_dram_parameter("z1d", [128, DK + 1], FR, isOutput=False)
    onesr = nc.declare_dram_parameter("onesr", [1, DK], FR, isOutput=False)
    onescols = nc.declare_dram_parameter("onescols", [128, NT], FR, isOutput=False)
    out = nc.declare_dram_parameter("attnT", [DK, S], FP, isOutput=True)
    dbg = None
    if debug_outputs:
        dbg = {
            "den": nc.declare_dram_parameter("den", [1, S], FP, isOutput=True).ap(),
            "num": nc.declare_dram_parameter("num", [DK, S], FP, isOutput=True).ap(),
            "qt": nc.declare_dram_parameter("qt", [DK, S], FP, isOutput=True).ap(),
            "kt": nc.declare_dram_parameter("kt", [DK, S], FP, isOutput=True).ap(),
            "vaug": nc.declare_dram_parameter("vaug", [128, NT * 65], FP, isOutput=True).ap(),
        }
    with tile.TileContext(nc) as tc:
        with ExitStack() as ctx:
            build_body(ctx, tc, xT.ap(), wqk.ap(), wv.ap(), misc.ap(), pmrow.ap(),
                       z1d.ap(), onesr.ap(), onescols.ap(), out.ap(), dbg)
    nc.finalize()
    _NC[debug_outputs] = nc
    return nc


def make_in_maps(x, padding_mask, Wq, bq, Wk, bk, Wv, bv):
    f32 = np.float32
    wqk_np = np.ascontiguousarray(
        np.concatenate([Wq, Wk], axis=1).reshape(DC, 128, 128).transpose(1, 0, 2)
    ).astype(f32)
    wv_np = np.ascontiguousarray(
        Wv.reshape(DC, 128, DK).transpose(1, 0, 2)
    ).astype(f32)
    # padding mask exactly as the reference: fp16 round-trip (0/1 -> exact)
    pmf = np.asarray(padding_mask).astype(np.float16).astype(f32)
    tri = (np.arange(128)[:, None] <= np.arange(128)[None, :]).astype(f32)
    eye = np.eye(128, dtype=f32)
    in_maps = []
    for b in range(B):
        misc = np.zeros((128, MISC_W), dtype=f32)
        misc[0:DK, MISC_BQK] = bq
        misc[DK:128, MISC_BQK] = bk
        misc[0:DK, MISC_BV] = bv
        misc[:, MISC_PMT : MISC_PMT + NT] = pmf[b].reshape(NT, 128).T
        misc[:, MISC_ID : MISC_ID + 128] = eye
        misc[:, MISC_TRI : MISC_TRI + 128] = tri
        z1d = np.zeros((128, DK + 1), dtype=f32)
        z1d[:, DK] = 1.0
        in_maps.append(
            {
                "z1d": z1d,
                "onesr": np.ones((1, DK), dtype=f32),
                "onescols": np.ones((128, NT), dtype=f32),
                "xT": np.ascontiguousarray(x[b].T).astype(f32),
                "wqk": wqk_np,
                "wv": wv_np,
                "misc": misc,
                "pmrow": np.ascontiguousarray(pmf[b : b + 1]),
            }
        )
    return in_maps


def kernel(**inputs) -> np.ndarray:
    nc = build_nc()
    in_maps = make_in_maps(
        inputs["x"], inputs["padding_mask"],
        inputs["Wq"], inputs["bq"], inputs["Wk"], inputs["bk"],
        inputs["Wv"], inputs["bv"],
    )
    res = run_bass_kernel_spmd(nc, in_maps, list(range(NCORES)))
    out = np.stack([res.results[b]["attnT"].T for b in range(B)])
    return np.ascontiguousarray(out.astype(np.float32))


if __name__ == "__main__":
    nc = build_nc()
    print("built ok; instructions:", len(nc.inst_map))


# revision 37
# speedup vs baseline: 1.1770x; 1.1770x over previous
"""Trainium2 Bass kernel for nn_AttentionHead_15805479649975.

Single attention head, B=8, S=2048, d_model=1024, d_k=64.
  Q = x@Wq+bq; K = x@Wk+bk; V = x@Wv+bv
  scores = Q K^T / 8; P = softmax(scores)            (full-row softmax)
  P = P * tril * (pm outer pm)                        (masks applied AFTER softmax)
  out = P V

Sharding: data-parallel over the batch dim — one batch element per NeuronCore,
projection weights replicated, no cross-core communication.

Per-core device algorithm (everything in fp32):
  Phase 1: stream xT (host-pretransposed x[b].T) in 8 d-chunks; compute
    QT/KT via one fused [Wq|Wk] matmul (M=128) and VT (M=64); PE-transpose
    VT into natural V chunks, scale by pm[k], append a ones column -> Vaug.
  Phase 2 (two q-halves of 1024 to fit PSUM): for each key tile j compute
    ST[k,q] = K Q^T (K=64 matmul), exp on ScalarE (scale=1/8 fused, no
    max-subtraction needed: |scores/8| is O(1)), then accumulate
      attnT[0:64, q] += V'[k,:]^T P^T[k, q]   (causal part, via Vaug)
      attnT[64,  q] += sum_k exp(..)          (softmax denominator, via the
                                               ones column / a zero|ones lhsT
                                               for the non-causal columns)
    Finally out[0:64] *= pm[q] / attnT[64] via reciprocal + gpsimd
    partition-broadcast, and DMA attnT back; host transposes to [S, 64].
"""

import math
from contextlib import ExitStack

import numpy as np

import concourse.bacc as bacc
import concourse.bass as bass
import concourse.mybir as mybir
import concourse.tile as tile
from concourse.bass_utils import run_bass_kernel_spmd

B, S, D, DK = 8, 2048, 1024, 64
NCORES = 8
FP = mybir.dt.float32
NT = S // 128          # 16 key tiles
NH = 2                 # q halves
HQ = S // NH           # 1024 queries per half
DC = D // 128          # 8 contraction chunks

# misc layout: col 0 = [bq; bk], col 1 = bv (rows 0:64), cols 2:18 = pmT,
# cols 18:146 = identity(128), cols 146:274 = tril mask (k<=q)
MISC_BQK = 0
MISC_BV = 1
MISC_PMT = 2
MISC_ID = 18
MISC_TRI = 146
MISC_W = 274

AF = mybir.ActivationFunctionType
MUL = mybir.AluOpType.mult

# float32r: single-pass "relaxed" fp32 matmul (TF32-ish). Plain fp32 runs as
# 2 half-speed passes (4x cycles/row) on TRN2's PE; fp32r streams at 1x.
USE_F32R = True
FR = mybir.dt.float32r if USE_F32R else mybir.dt.float32
BF = mybir.dt.bfloat16


def _mmcast(ap):
    return ap


def _chunks(lo, hi, step=512):
    """Split [lo, hi) at the 512-column grid (PSUM bank boundaries).

    Bank alignment matters: a matmul with start=True clears the has_written
    bits of every bank it touches, so no chunk may straddle a bank whose
    prior accumulations must survive."""
    out = []
    while lo < hi:
        nxt = min((lo // step + 1) * step, hi)
        out.append((lo, nxt))
        lo = nxt
    return out


def build_body(ctx: ExitStack, tc, xT, wqk, wv, misc, pmrow, z1d, onesr, onescols, out, dbg=None):
    nc = tc.nc

    consts = ctx.enter_context(tc.tile_pool(name="consts", bufs=1))
    xpool = ctx.enter_context(tc.tile_pool(name="x", bufs=DC))
    proj = ctx.enter_context(tc.tile_pool(name="proj", bufs=1))
    ppool = ctx.enter_context(tc.tile_pool(name="pt", bufs=3))
    dpool = ctx.enter_context(tc.tile_pool(name="ptd", bufs=2))
    fin = ctx.enter_context(tc.tile_pool(name="fin", bufs=2))

    # ---- constants (weights first; everything not needed until later is
    # DMA'd after the x chunks so x transfers start as early as possible)
    wqk_s = consts.tile([128, DC, 128], BF)
    nc.sync.dma_start(wqk_s[:], wqk[:])
    wv_s = consts.tile([128, DC, DK], BF)
    nc.sync.dma_start(wv_s[:], wv[:])

    # ---- x chunks (all resident; DMA-bound phase)
    xs = []
    for c in range(DC):
        xc = xpool.tile([128, S], BF, tag="xc")
        nc.sync.dma_start(xc[:], xT[c * 128 : (c + 1) * 128, :])
        xs.append(xc)

    misc_s = consts.tile([128, MISC_W], FP)
    nc.sync.dma_start(misc_s[:], misc[:])
    pmrow_s = consts.tile([1, S], FP)
    nc.sync.dma_start(pmrow_s[:], pmrow[:])
    z1_s = consts.tile([128, DK + 1], FR)
    nc.sync.dma_start(z1_s[:], z1d[:])
    ones_row = consts.tile([1, DK], FR)
    nc.sync.dma_start(ones_row[:], onesr[:])

    qt_s = proj.tile([DK, S], FR, tag="qt")
    kt_s = proj.tile([DK, S], FR, tag="kt")
    vt_s = proj.tile([DK, S], FP, tag="vt")
    vaug_s = proj.tile([128, NT * (DK + 1)], FR, tag="vaug")
    attn_s = proj.tile([DK, S], FP, tag="attn")
    # ones columns of vaug (one strided DMA; memset can't write f32r)
    vaug_cols = vaug_s[:].rearrange("p (t c) -> p t c", c=DK + 1)[:, :, DK : DK + 1]
    nc.sync.dma_start(vaug_cols, onescols[:].rearrange("p (t o) -> p t o", o=1))

    ident = misc_s[0:DK, MISC_ID : MISC_ID + DK]

    # ---- phase 1: projections
    with tc.tile_pool(name="p1", bufs=2, space="PSUM") as p1:
        for n in range(4):
            nlo = n * 512
            qk_ps = p1.tile([128, 512], FP, tag="qk")
            for c in range(DC):
                nc.tensor.matmul(
                    qk_ps[:], _mmcast(wqk_s[:, c, :]),
                    _mmcast(xs[c][:, nlo : nlo + 512]),
                    start=(c == 0), stop=(c == DC - 1),
                )
            nc.vector.tensor_scalar_add(
                qt_s[:, nlo : nlo + 512], qk_ps[0:DK, :],
                misc_s[0:DK, MISC_BQK : MISC_BQK + 1],
            )
            nc.vector.tensor_scalar_add(
                kt_s[:, nlo : nlo + 512], qk_ps[DK:128, :],
                misc_s[DK:128, MISC_BQK : MISC_BQK + 1],
            )
            v_ps = p1.tile([DK, 512], FP, tag="v")
            for c in range(DC):
                nc.tensor.matmul(
                    v_ps[:], _mmcast(wv_s[:, c, :]),
                    _mmcast(xs[c][:, nlo : nlo + 512]),
                    start=(c == 0), stop=(c == DC - 1),
                )
            nc.vector.tensor_scalar_add(
                vt_s[:, nlo : nlo + 512], v_ps[:],
                misc_s[0:DK, MISC_BV : MISC_BV + 1],
            )
            # VT -> V natural chunks for this quarter, scaled by pm[k]
            for j in range(4 * n, 4 * n + 4):
                tr_ps = p1.tile([128, DK], FP, tag="tr")
                nc.tensor.transpose(tr_ps[:], vt_s[:, j * 128 : (j + 1) * 128], ident)
                nc.vector.tensor_scalar_mul(
                    vaug_s[:, j * 65 : j * 65 + DK], tr_ps[:],
                    misc_s[:, MISC_PMT + j : MISC_PMT + j + 1],
                )

    # ---- phase 2: scores/softmax/PV per q-half
    with (
        tc.tile_pool(name="p2st", bufs=2, space="PSUM") as p2st,
        tc.tile_pool(name="p2att", bufs=1, space="PSUM") as p2att,
    ):
        for h in range(NH):
            qlo = h * HQ
            att_ps = p2att.tile([DK + 1, HQ], FP, tag="att")
            bank_cleared = [False] * (HQ // 512)

            def pv_mm(lo, hi, lhsT, rhs, part_hi, stop=False, rhs_off=0):
                for clo, chi in _chunks(lo, hi):
                    bank = clo // 512
                    nc.tensor.matmul(
                        att_ps[0:part_hi, clo:chi], _mmcast(lhsT),
                        _mmcast(rhs[:, clo - rhs_off : chi - rhs_off]),
                        start=not bank_cleared[bank], stop=stop,
                    )
                    bank_cleared[bank] = True

            for j in range(NT):
                st_ps = p2st.tile([128, HQ], FP, tag="st")
                for lo, hi in _chunks(0, HQ):
                    nc.tensor.matmul(
                        st_ps[:, lo:hi],
                        _mmcast(kt_s[:, j * 128 : (j + 1) * 128]),
                        _mmcast(qt_s[:, qlo + lo : qlo + hi]),
                        start=True, stop=True,
                    )
                pt = ppool.tile([128, HQ], FR, tag="pt")
                nc.scalar.activation(pt[:], st_ps[:], AF.Exp, scale=1.0 / math.sqrt(DK))

                split = min(max(128 * (j + 1) - qlo, 0), HQ)
                # diagonal block: tril-masked V-part (emitted first so it owns
                # the bank-clear when it is the first writer)
                dlo = 128 * j - qlo
                if 0 <= dlo < HQ:
                    ptd = dpool.tile([128, 128], FR, tag="ptd")
                    nc.vector.tensor_tensor(
                        ptd[:], pt[:, dlo : dlo + 128].bitcast(FP),
                        misc_s[:, MISC_TRI : MISC_TRI + 128], MUL,
                    )
                    pv_mm(dlo, dlo + 128, vaug_s[:, j * 65 : j * 65 + DK], ptd,
                          part_hi=DK, rhs_off=dlo)
                # denominator for columns q < 128*(j+1): zero|ones lhsT
                pv_mm(0, split, z1_s[:], pt, part_hi=DK + 1, stop=(j == NT - 1))
                # strictly-causal columns: augmented [pm*V | 1]
                pv_mm(split, HQ, vaug_s[:, j * 65 : (j + 1) * 65], pt,
                      part_hi=DK + 1)
            # tail: out = attnT[0:64] * (pm[q] / denom[q])
            rcp = fin.tile([1, HQ], FP, tag="rcp")
            nc.vector.reciprocal(rcp[:], att_ps[DK : DK + 1, :])
            rfin = fin.tile([1, HQ], FR, tag="rfin")
            nc.vector.tensor_tensor(rfin[:], rcp[:], pmrow_s[:, qlo : qlo + HQ], MUL)
            # broadcast rfin across 64 partitions: K=1 outer product on PE
            bc_full = p2st.tile([128, HQ], FP, tag="st")
            bc_ps = bc_full[0:DK, :]
            for lo, hi in _chunks(0, HQ):
                nc.tensor.matmul(
                    bc_ps[:, lo:hi], _mmcast(ones_row[:]), _mmcast(rfin[:, lo:hi]),
                    start=True, stop=True,
                )
            bc_s = fin.tile([DK, HQ], FP, tag="bc")
            nc.vector.tensor_scalar_mul(bc_s[:], bc_ps[:], 1.0)
            nc.vector.tensor_tensor(
                attn_s[:, qlo : qlo + HQ], att_ps[0:DK, :], bc_s[:], MUL
            )
            nc.sync.dma_start(out[:, qlo : qlo + HQ], attn_s[:, qlo : qlo + HQ])
            if dbg is not None:
                den_s = fin.tile([1, HQ], FP, tag="dens")
                nc.vector.tensor_scalar_mul(den_s[:], att_ps[DK : DK + 1, :], 1.0)
                nc.sync.dma_start(dbg["den"][:, qlo : qlo + HQ], den_s[:])
                num_s = fin.tile([DK, HQ], FP, tag="nums")
                nc.vector.tensor_scalar_mul(num_s[:], att_ps[0:DK, :], 1.0)
                nc.sync.dma_start(dbg["num"][:, qlo : qlo + HQ], num_s[:])
    if dbg is not None:
        nc.sync.dma_start(dbg["qt"][:], qt_s[:].bitcast(FP))
        nc.sync.dma_start(dbg["kt"][:], kt_s[:].bitcast(FP))
        nc.sync.dma_start(dbg["vaug"][:], vaug_s[:].bitcast(FP))


_NC = {}


def build_nc(debug_outputs=False):
    if debug_outputs in _NC:
        return _NC[debug_outputs]
    nc = bacc.Bacc("TRN2", target_bir_lowering=False, debug=False, num_devices=NCORES)
    xT = nc.declare_dram_parameter("xT", [D, S], BF, isOutput=False)
    wqk = nc.declare_dram_parameter("wqk", [128, DC, 128], BF, isOutput=False)
    wv = nc.declare_dram_parameter("wv", [128, DC, DK], BF, isOutput=False)
    misc = nc.declare_dram_parameter("misc", [128, MISC_W], FP, isOutput=False)
    pmrow = nc.declare_dram_parameter("pmrow", [1, S], FP, isOutput=False)
    z1d = nc.declare_dram_parameter("z1d", [128, DK + 1], FR, isOutput=False)
    onesr = nc.declare_dram_parameter("onesr", [1, DK], FR, isOutput=False)
    onescols = nc.declare_dram_parameter("onescols", [128, NT], FR, isOutput=False)
    out = nc.declare_dram_parameter("attnT", [DK, S], FP, isOutput=True)
    dbg = None
    if debug_outputs:
        dbg = {
            "den": nc.declare_dram_parameter("den", [1, S], FP, isOutput=True).ap(),
            "num": nc.declare_dram_parameter("num", [DK, S], FP, isOutput=True).ap(),
            "qt": nc.declare_dram_parameter("qt", [DK, S], FP, isOutput=True).ap(),
            "kt": nc.declare_dram_parameter("kt", [DK, S], FP, isOutput=True).ap(),
            "vaug": nc.declare_dram_parameter("vaug", [128, NT * 65], FP, isOutput=True).ap(),
        }
    with tile.TileContext(nc) as tc:
        with ExitStack() as ctx:
            build_body(ctx, tc, xT.ap(), wqk.ap(), wv.ap(), misc.ap(), pmrow.ap(),
                       z1d.ap(), onesr.ap(), onescols.ap(), out.ap(), dbg)
    nc.finalize()
    _NC[debug_outputs] = nc
    return nc


def make_in_maps(x, padding_mask, Wq, bq, Wk, bk, Wv, bv):
    f32 = np.float32
    import ml_dtypes
    bf16 = ml_dtypes.bfloat16
    wqk_np = np.ascontiguousarray(
        np.concatenate([Wq, Wk], axis=1).reshape(DC, 128, 128).transpose(1, 0, 2)
    ).astype(bf16)
    wv_np = np.ascontiguousarray(
        Wv.reshape(DC, 128, DK).transpose(1, 0, 2)
    ).astype(bf16)
    # padding mask exactly as the reference: fp16 round-trip (0/1 -> exact)
    pmf = np.asarray(padding_mask).astype(np.float16).astype(f32)
    tri = (np.arange(128)[:, None] <= np.arange(128)[None, :]).astype(f32)
    eye = np.eye(128, dtype=f32)
    in_maps = []
    for b in range(B):
        misc = np.zeros((128, MISC_W), dtype=f32)
        misc[0:DK, MISC_BQK] = bq
        misc[DK:128, MISC_BQK] = bk
        misc[0:DK, MISC_BV] = bv
        misc[:, MISC_PMT : MISC_PMT + NT] = pmf[b].reshape(NT, 128).T
        misc[:, MISC_ID : MISC_ID + 128] = eye
        misc[:, MISC_TRI : MISC_TRI + 128] = tri
        z1d = np.zeros((128, DK + 1), dtype=f32)
        z1d[:, DK] = 1.0
        in_maps.append(
            {
                "z1d": z1d,
                "onesr": np.ones((1, DK), dtype=f32),
                "onescols": np.ones((128, NT), dtype=f32),
                "xT": np.ascontiguousarray(x[b].T).astype(bf16),
                "wqk": wqk_np,
                "wv": wv_np,
                "misc": misc,
                "pmrow": np.ascontiguousarray(pmf[b : b + 1]),
            }
        )
    return in_maps


def kernel(**inputs) -> np.ndarray:
    nc = build_nc()
    in_maps = make_in_maps(
        inputs["x"], inputs["padding_mask"],
        inputs["Wq"], inputs["bq"], inputs["Wk"], inputs["bk"],
        inputs["Wv"], inputs["bv"],
    )
    res = run_bass_kernel_spmd(nc, in_maps, list(range(NCORES)))
    out = np.stack([res.results[b]["attnT"].T for b in range(B)])
    return np.ascontiguousarray(out.astype(np.float32))


if __name__ == "__main__":
    nc = build_nc()
    print("built ok; instructions:", len(nc.inst_map))


# revision 39
# speedup vs baseline: 1.2047x; 1.0235x over previous
"""Trainium2 Bass kernel for nn_AttentionHead_15805479649975.

Single attention head, B=8, S=2048, d_model=1024, d_k=64.
  Q = x@Wq+bq; K = x@Wk+bk; V = x@Wv+bv
  scores = Q K^T / 8; P = softmax(scores)            (full-row softmax)
  P = P * tril * (pm outer pm)                        (masks applied AFTER softmax)
  out = P V

Sharding: data-parallel over the batch dim — one batch element per NeuronCore,
projection weights replicated, no cross-core communication.

Per-core device algorithm (everything in fp32):
  Phase 1: stream xT (host-pretransposed x[b].T) in 8 d-chunks; compute
    QT/KT via one fused [Wq|Wk] matmul (M=128) and VT (M=64); PE-transpose
    VT into natural V chunks, scale by pm[k], append a ones column -> Vaug.
  Phase 2 (two q-halves of 1024 to fit PSUM): for each key tile j compute
    ST[k,q] = K Q^T (K=64 matmul), exp on ScalarE (scale=1/8 fused, no
    max-subtraction needed: |scores/8| is O(1)), then accumulate
      attnT[0:64, q] += V'[k,:]^T P^T[k, q]   (causal part, via Vaug)
      attnT[64,  q] += sum_k exp(..)          (softmax denominator, via the
                                               ones column / a zero|ones lhsT
                                               for the non-causal columns)
    Finally out[0:64] *= pm[q] / attnT[64] via reciprocal + gpsimd
    partition-broadcast, and DMA attnT back; host transposes to [S, 64].
"""

import math
from contextlib import ExitStack

import numpy as np

import concourse.bacc as bacc
import concourse.bass as bass
import concourse.mybir as mybir
import concourse.tile as tile
from concourse.bass_utils import run_bass_kernel_spmd

B, S, D, DK = 8, 2048, 1024, 64
NCORES = 8
FP = mybir.dt.float32
NT = S // 128          # 16 key tiles
NH = 2                 # q halves
HQ = S // NH           # 1024 queries per half
DC = D // 128          # 8 contraction chunks

# misc layout: col 0 = [bq; bk], col 1 = bv (rows 0:64), cols 2:18 = pmT,
# cols 18:146 = identity(128), cols 146:274 = tril mask (k<=q)
MISC_BQK = 0
MISC_BV = 1
MISC_PMT = 2
MISC_ID = 18
MISC_TRI = 146
MISC_W = 274

AF = mybir.ActivationFunctionType
MUL = mybir.AluOpType.mult

# float32r: single-pass "relaxed" fp32 matmul (TF32-ish). Plain fp32 runs as
# 2 half-speed passes (4x cycles/row) on TRN2's PE; fp32r streams at 1x.
USE_F32R = True
FR = mybir.dt.float32r if USE_F32R else mybir.dt.float32
BF = mybir.dt.bfloat16


def _mmcast(ap):
    return ap


def _chunks(lo, hi, step=512):
    """Split [lo, hi) at the 512-column grid (PSUM bank boundaries).

    Bank alignment matters: a matmul with start=True clears the has_written
    bits of every bank it touches, so no chunk may straddle a bank whose
    prior accumulations must survive."""
    out = []
    while lo < hi:
        nxt = min((lo // step + 1) * step, hi)
        out.append((lo, nxt))
        lo = nxt
    return out


def build_body(ctx: ExitStack, tc, xT, wqk, wv, misc, pmrow, z1d, onesr, onescols, out, dbg=None):
    nc = tc.nc

    consts = ctx.enter_context(tc.tile_pool(name="consts", bufs=1))
    xpool = ctx.enter_context(tc.tile_pool(name="x", bufs=DC))
    proj = ctx.enter_context(tc.tile_pool(name="proj", bufs=1))
    ppool = ctx.enter_context(tc.tile_pool(name="pt", bufs=5))
    dpool = ctx.enter_context(tc.tile_pool(name="ptd", bufs=2))
    fin = ctx.enter_context(tc.tile_pool(name="fin", bufs=2))

    # ---- constants (weights first; everything not needed until later is
    # DMA'd after the x chunks so x transfers start as early as possible)
    wqk_s = consts.tile([128, DC, 128], BF)
    nc.sync.dma_start(wqk_s[:], wqk[:])
    wv_s = consts.tile([128, DC, DK], BF)
    nc.sync.dma_start(wv_s[:], wv[:])

    # ---- x chunks (all resident; DMA-bound phase)
    xs = []
    for c in range(DC):
        xc = xpool.tile([128, S], BF, tag="xc")
        nc.sync.dma_start(xc[:], xT[c * 128 : (c + 1) * 128, :])
        xs.append(xc)

    misc_s = consts.tile([128, MISC_W], FP)
    nc.sync.dma_start(misc_s[:], misc[:])
    pmrow_s = consts.tile([1, S], FP)
    nc.sync.dma_start(pmrow_s[:], pmrow[:])
    z1_s = consts.tile([128, DK + 1], FR)
    nc.sync.dma_start(z1_s[:], z1d[:])
    ones_row = consts.tile([1, DK], FR)
    nc.sync.dma_start(ones_row[:], onesr[:])

    qt_s = proj.tile([DK, S], FR, tag="qt")
    kt_s = proj.tile([DK, S], FR, tag="kt")
    vt_s = proj.tile([DK, S], FP, tag="vt")
    vaug_s = proj.tile([128, NT * (DK + 1)], FR, tag="vaug")
    attn_s = proj.tile([DK, S], FP, tag="attn")
    # ones columns of vaug (one strided DMA; memset can't write f32r)
    vaug_cols = vaug_s[:].rearrange("p (t c) -> p t c", c=DK + 1)[:, :, DK : DK + 1]
    nc.sync.dma_start(vaug_cols, onescols[:].rearrange("p (t o) -> p t o", o=1))

    ident = misc_s[0:DK, MISC_ID : MISC_ID + DK]

    # ---- phase 1: projections
    with tc.tile_pool(name="p1", bufs=2, space="PSUM") as p1:
        for n in range(4):
            nlo = n * 512
            qk_ps = p1.tile([128, 512], FP, tag="qk")
            for c in range(DC):
                nc.tensor.matmul(
                    qk_ps[:], _mmcast(wqk_s[:, c, :]),
                    _mmcast(xs[c][:, nlo : nlo + 512]),
                    start=(c == 0), stop=(c == DC - 1),
                )
            nc.vector.tensor_scalar_add(
                qt_s[:, nlo : nlo + 512], qk_ps[0:DK, :],
                misc_s[0:DK, MISC_BQK : MISC_BQK + 1],
            )
            nc.vector.tensor_scalar_add(
                kt_s[:, nlo : nlo + 512], qk_ps[DK:128, :],
                misc_s[DK:128, MISC_BQK : MISC_BQK + 1],
            )
            v_ps = p1.tile([DK, 512], FP, tag="v")
            for c in range(DC):
                nc.tensor.matmul(
                    v_ps[:], _mmcast(wv_s[:, c, :]),
                    _mmcast(xs[c][:, nlo : nlo + 512]),
                    start=(c == 0), stop=(c == DC - 1),
                )
            nc.vector.tensor_scalar_add(
                vt_s[:, nlo : nlo + 512], v_ps[:],
                misc_s[0:DK, MISC_BV : MISC_BV + 1],
            )
            # VT -> V natural chunks for this quarter, scaled by pm[k]
            for j in range(4 * n, 4 * n + 4):
                tr_ps = p1.tile([128, DK], FP, tag="tr")
                nc.tensor.transpose(tr_ps[:], vt_s[:, j * 128 : (j + 1) * 128], ident)
                nc.vector.tensor_scalar_mul(
                    vaug_s[:, j * 65 : j * 65 + DK], tr_ps[:],
                    misc_s[:, MISC_PMT + j : MISC_PMT + j + 1],
                )

    # ---- phase 2: scores/softmax/PV per q-half. Both halves' j-loops are
    # emitted before either normalization tail: the tail's broadcast matmul
    # waits on a DVE chain, and emitting it mid-stream stalls the in-order
    # PE ahead of the second half's score matmuls.
    with (
        tc.tile_pool(name="p2st", bufs=2, space="PSUM") as p2st,
        tc.tile_pool(name="p2att", bufs=2, space="PSUM") as p2att,
    ):
        att_tiles = []
        for h in range(NH):
            qlo = h * HQ
            att_ps = p2att.tile([DK + 1, HQ], FP, tag="att")
            att_tiles.append(att_ps)
            bank_cleared = [False] * (HQ // 512)

            def pv_mm(lo, hi, lhsT, rhs, part_hi, stop=False, rhs_off=0):
                for clo, chi in _chunks(lo, hi):
                    bank = clo // 512
                    nc.tensor.matmul(
                        att_ps[0:part_hi, clo:chi], _mmcast(lhsT),
                        _mmcast(rhs[:, clo - rhs_off : chi - rhs_off]),
                        start=not bank_cleared[bank], stop=stop,
                    )
                    bank_cleared[bank] = True

            for j in range(NT):
                st_ps = p2st.tile([128, HQ], FP, tag="st")
                for lo, hi in _chunks(0, HQ):
                    nc.tensor.matmul(
                        st_ps[:, lo:hi],
                        _mmcast(kt_s[:, j * 128 : (j + 1) * 128]),
                        _mmcast(qt_s[:, qlo + lo : qlo + hi]),
                        start=True, stop=True,
                    )
                pt = ppool.tile([128, HQ], FR, tag="pt")
                nc.scalar.activation(pt[:], st_ps[:], AF.Exp, scale=1.0 / math.sqrt(DK))

                split = min(max(128 * (j + 1) - qlo, 0), HQ)
                # diagonal block: tril-masked V-part (emitted first so it owns
                # the bank-clear when it is the first writer)
                dlo = 128 * j - qlo
                if 0 <= dlo < HQ:
                    ptd = dpool.tile([128, 128], FR, tag="ptd")
                    nc.vector.tensor_tensor(
                        ptd[:], pt[:, dlo : dlo + 128].bitcast(FP),
                        misc_s[:, MISC_TRI : MISC_TRI + 128], MUL,
                    )
                    pv_mm(dlo, dlo + 128, vaug_s[:, j * 65 : j * 65 + DK], ptd,
                          part_hi=DK, rhs_off=dlo)
                # denominator for columns q < 128*(j+1): zero|ones lhsT
                pv_mm(0, split, z1_s[:], pt, part_hi=DK + 1, stop=(j == NT - 1))
                # strictly-causal columns: augmented [pm*V | 1]
                pv_mm(split, HQ, vaug_s[:, j * 65 : (j + 1) * 65], pt,
                      part_hi=DK + 1)
        for h in range(NH):
            qlo = h * HQ
            att_ps = att_tiles[h]
            # tail: out = attnT[0:64] * (pm[q] / denom[q])
            rcp = fin.tile([1, HQ], FP, tag="rcp")
            nc.vector.reciprocal(rcp[:], att_ps[DK : DK + 1, :])
            rfin = fin.tile([1, HQ], FR, tag="rfin")
            nc.vector.tensor_tensor(rfin[:], rcp[:], pmrow_s[:, qlo : qlo + HQ], MUL)
            # broadcast rfin across 64 partitions: K=1 outer product on PE
            bc_full = p2st.tile([128, HQ], FP, tag="st")
            bc_ps = bc_full[0:DK, :]
            for lo, hi in _chunks(0, HQ):
                nc.tensor.matmul(
                    bc_ps[:, lo:hi], _mmcast(ones_row[:]), _mmcast(rfin[:, lo:hi]),
                    start=True, stop=True,
                )
            bc_s = fin.tile([DK, HQ], FP, tag="bc")
            nc.vector.tensor_scalar_mul(bc_s[:], bc_ps[:], 1.0)
            nc.vector.tensor_tensor(
                attn_s[:, qlo : qlo + HQ], att_ps[0:DK, :], bc_s[:], MUL
            )
            nc.sync.dma_start(out[:, qlo : qlo + HQ], attn_s[:, qlo : qlo + HQ])
            if dbg is not None:
                den_s = fin.tile([1, HQ], FP, tag="dens")
                nc.vector.tensor_scalar_mul(den_s[:], att_ps[DK : DK + 1, :], 1.0)
                nc.sync.dma_start(dbg["den"][:, qlo : qlo + HQ], den_s[:])
                num_s = fin.tile([DK, HQ], FP, tag="nums")
                nc.vector.tensor_scalar_mul(num_s[:], att_ps[0:DK, :], 1.0)
                nc.sync.dma_start(dbg["num"][:, qlo : qlo + HQ], num_s[:])
    if dbg is not None:
        nc.sync.dma_start(dbg["qt"][:], qt_s[:].bitcast(FP))
        nc.sync.dma_start(dbg["kt"][:], kt_s[:].bitcast(FP))
        nc.sync.dma_start(dbg["vaug"][:], vaug_s[:].bitcast(FP))


_NC = {}


def build_nc(debug_outputs=False):
    if debug_outputs in _NC:
        return _NC[debug_outputs]
    nc = bacc.Bacc("TRN2", target_bir_lowering=False, debug=False, num_devices=NCORES)
    xT = nc.declare_dram_parameter("xT", [D, S], BF, isOutput=False)
    wqk = nc.declare_dram_parameter("wqk", [128, DC, 128], BF, isOutput=False)
    wv = nc.declare_dram_parameter("wv", [128, DC, DK], BF, isOutput=False)
    misc = nc.declare_dram_parameter("misc", [128, MISC_W], FP, isOutput=False)
    pmrow = nc.declare_dram_parameter("pmrow", [1, S], FP, isOutput=False)
    z1d = nc.declare_dram_parameter("z1d", [128, DK + 1], FR, isOutput=False)
    onesr = nc.declare_dram_parameter("onesr", [1, DK], FR, isOutput=False)
    onescols = nc.declare_dram_parameter("onescols", [128, NT], FR, isOutput=False)
    out = nc.declare_dram_parameter("attnT", [DK, S], FP, isOutput=True)
    dbg = None
    if debug_outputs:
        dbg = {
            "den": nc.declare_dram_parameter("den", [1, S], FP, isOutput=True).ap(),
            "num": nc.declare_dram_parameter("num", [DK, S], FP, isOutput=True).ap(),
            "qt": nc.declare_dram_parameter("qt", [DK, S], FP, isOutput=True).ap(),
            "kt": nc.declare_dram_parameter("kt", [DK, S], FP, isOutput=True).ap(),
            "vaug": nc.declare_dram_parameter("vaug", [128, NT * 65], FP, isOutput=True).ap(),
        }
    with tile.TileContext(nc) as tc:
        with ExitStack() as ctx:
            build_body(ctx, tc, xT.ap(), wqk.ap(), wv.ap(), misc.ap(), pmrow.ap(),
                       z1d.ap(), onesr.ap(), onescols.ap(), out.ap(), dbg)
    nc.finalize()
    _NC[debug_outputs] = nc
    return nc


def make_in_maps(x, padding_mask, Wq, bq, Wk, bk, Wv, bv):
    f32 = np.float32
    import ml_dtypes
    bf16 = ml_dtypes.bfloat16
    wqk_np = np.ascontiguousarray(
        np.concatenate([Wq, Wk], axis=1).reshape(DC, 128, 128).transpose(1, 0, 2)
    ).astype(bf16)
    wv_np = np.ascontiguousarray(
        Wv.reshape(DC, 128, DK).transpose(1, 0, 2)
    ).astype(bf16)
    # padding mask exactly as the reference: fp16 round-trip (0/1 -> exact)
    pmf = np.asarray(padding_mask).astype(np.float16).astype(f32)
    tri = (np.arange(128)[:, None] <= np.arange(128)[None, :]).astype(f32)
    eye = np.eye(128, dtype=f32)
    in_maps = []
    for b in range(B):
        misc = np.zeros((128, MISC_W), dtype=f32)
        misc[0:DK, MISC_BQK] = bq
        misc[DK:128, MISC_BQK] = bk
        misc[0:DK, MISC_BV] = bv
        misc[:, MISC_PMT : MISC_PMT + NT] = pmf[b].reshape(NT, 128).T
        misc[:, MISC_ID : MISC_ID + 128] = eye
        misc[:, MISC_TRI : MISC_TRI + 128] = tri
        z1d = np.zeros((128, DK + 1), dtype=f32)
        z1d[:, DK] = 1.0
        in_maps.append(
            {
                "z1d": z1d,
                "onesr": np.ones((1, DK), dtype=f32),
                "onescols": np.ones((128, NT), dtype=f32),
                "xT": np.ascontiguousarray(x[b].T).astype(bf16),
                "wqk": wqk_np,
                "wv": wv_np,
                "misc": misc,
                "pmrow": np.ascontiguousarray(pmf[b : b + 1]),
            }
        )
    return in_maps


def kernel(**inputs) -> np.ndarray:
    nc = build_nc()
    in_maps = make_in_maps(
        inputs["x"], inputs["padding_mask"],
        inputs["Wq"], inputs["bq"], inputs["Wk"], inputs["bk"],
        inputs["Wv"], inputs["bv"],
    )
    res = run_bass_kernel_spmd(nc, in_maps, list(range(NCORES)))
    out = np.stack([res.results[b]["attnT"].T for b in range(B)])
    return np.ascontiguousarray(out.astype(np.float32))


if __name__ == "__main__":
    nc = build_nc()
    print("built ok; instructions:", len(nc.inst_map))
